# revision 63
# baseline (speedup 1.0000x reference)
"""Multi-head attention on 8 Trainium2 NeuronCores (v2, fp16 + flipped attnV).

Sharding: data-parallel over batch (4) x tensor-parallel over head-groups (2).
Core c handles batch c//2, heads [8*(c%2), 8*(c%2)+8). Each core computes its
partial out-projection (over its 512 channels); host sums the pair per batch.

Device-side design (per core, all values fp16; PSUM accumulation f32):
  Q^T, K^T  [512, 2048] channel-major (W chunks stationary)
  V         [2048, 8*65] token-major (65 cols/head: 64 V + ones column)
  scores^T[s,t] in PSUM chunks A [128,2048] / B [128,1024] (pattern 4,2,4,2,4)
  exp on ACT (scale=1/8) -> P fp16 in SBUF; mask-mul on DVE (fp16 2x mode)
  attnV flipped: lhsT = P^T chunk [128s,128t], rhs = V[s,65] -> acc[t, 4*65]
  normalize: DVE recip of denom col + Pool tensor_scalar -> attn_n [t, 4*64]
  PE transpose (identity) -> tT psum fp16 [d, t] -> Pool copy -> onrm
  out-proj: lhsT = Wo^T chunks, rhs = onrm -> op psum -> Pool copy -> DMA out
"""
import sys

sys.path.insert(0, "/opt/trn_rl_repo")

import numpy as np

import concourse.bass as bass
import concourse.mybir as mybir
import concourse.tile as tile
from concourse import bacc
from concourse.bass_utils import run_bass_kernel_spmd

D_MODEL = 1024
NUM_HEADS = 16
DK = 64
B, S = 4, 2048
NCORES = 8
OG = 512            # channels per head-group
HPG = 8             # heads per group
IC = D_MODEL // 128  # 8 contraction chunks
F32 = mybir.dt.float32
F16 = mybir.dt.float16

# scores: 8 chunks of 2 s-chunks each on a 3-deep psum ring S0/S1/S2
# (shared with out-proj accumulators). PSUM banks: 3*2 + acc=1 + tT=1 = 8.
NCHUNK = 8


def build_module():
    nc = bacc.Bacc("TRN2", target_bir_lowering=False, debug=False,
                   num_devices=NCORES)
    XQT = nc.dram_tensor("XQT", [D_MODEL, S], F16, kind="ExternalInput").ap()
    XKT = nc.dram_tensor("XKT", [D_MODEL, S], F16, kind="ExternalInput").ap()
    XVT = nc.dram_tensor("XVT", [D_MODEL, S], F16, kind="ExternalInput").ap()
    WQT = nc.dram_tensor("WQT", [D_MODEL, OG], F16, kind="ExternalInput").ap()
    WKT = nc.dram_tensor("WKT", [D_MODEL, OG], F16, kind="ExternalInput").ap()
    WVT = nc.dram_tensor("WVT", [D_MODEL, OG], F16, kind="ExternalInput").ap()
    WOT = nc.dram_tensor("WOT", [OG, D_MODEL], F16, kind="ExternalInput").ap()
    MASKT = nc.dram_tensor("MASKT", [S, S], F16, kind="ExternalInput").ap()
    BQ = nc.dram_tensor("BQ", [OG], F32, kind="ExternalInput").ap()
    BK = nc.dram_tensor("BK", [OG], F32, kind="ExternalInput").ap()
    BV = nc.dram_tensor("BV", [1, OG], F16, kind="ExternalInput").ap()
    IDT = nc.dram_tensor("IDT", [128, 128], F16, kind="ExternalInput").ap()
    OUTT = nc.dram_tensor("OUTT", [D_MODEL, S], F32, kind="ExternalOutput").ap()

    Exp = mybir.ActivationFunctionType.Exp

    with tile.TileContext(nc) as tc:
        with tc.tile_pool(name="persist", bufs=1) as pp, \
             tc.tile_pool(name="qkpool", bufs=1) as qkp:
            # projection outputs, resident through the whole kernel.
            # Q is per-(och, t-block) so late Q rounds create no false deps.
            qtt = {(och, t): qkp.tile([128, 512], F16, name=f"qt{och}_{t}")
                   for och in range(4) for t in range(4)}
            kt = [qkp.tile([128, S], F16, name=f"kt{j}") for j in range(4)]
            bq_t = pp.tile([128, 4], F32, name="bq_t")
            bk_t = pp.tile([128, 4], F32, name="bk_t")
            bv_t = pp.tile([1, OG], F16, name="bv_t")
            id_t = pp.tile([128, 128], F16, name="id_t")
            ones_t = pp.tile([1, 128], F16, name="ones_t")

            nc.sync.dma_start(out=bq_t[:], in_=BQ.rearrange("(j p) -> p j", p=128))
            nc.sync.dma_start(out=bk_t[:], in_=BK.rearrange("(j p) -> p j", p=128))
            nc.sync.dma_start(out=bv_t[:], in_=BV)
            nc.sync.dma_start(out=id_t[:], in_=IDT)
            nc.vector.memset(ones_t[:], 1.0)

            # ---------------- Phase A: projections ----------------
            vtp = tc.alloc_tile_pool(name="vtpool", bufs=1)
            vt = [vtp.tile([128, HPG * 65], F16, name=f"vt{j}")
                  for j in range(16)]
            wp = tc.alloc_tile_pool(name="wpool", bufs=1)
            xp = tc.alloc_tile_pool(name="xpool", bufs=4)
            psA = tc.alloc_tile_pool(name="psA", bufs=1, space="PSUM")
            wq = [wp.tile([128, OG], F16, name=f"wq{i}") for i in range(IC)]
            wk = [wp.tile([128, OG], F16, name=f"wk{i}") for i in range(IC)]
            wv = [wp.tile([128, OG], F16, name=f"wv{i}") for i in range(IC)]

            # K^T, then V (aliases wq tags), then Q t-block 0 only; Q's
            # remaining t-blocks stream through the phase-B psum ring.
            engs = (nc.sync, nc.scalar, nc.gpsimd)

            def load_w(wdram, wts):
                for i in range(IC):
                    (nc.sync if i % 2 else nc.gpsimd).dma_start(
                        out=wts[i][:], in_=wdram[i * 128:(i + 1) * 128, :])

            def load_xtb(xdram, tb):
                """one t-block [128,512] per contraction chunk, ring tags."""
                xts = []
                for i in range(IC):
                    xt = xp.tile([128, 512], F16, tag=f"x{i}",
                                 name=f"x{i}_{tb}")
                    (nc.sync if i % 2 else nc.gpsimd).dma_start(
                        out=xt[:],
                        in_=xdram[i * 128:(i + 1) * 128,
                                  tb * 512:(tb + 1) * 512])
                    xts.append(xt)
                return xts

            load_w(WKT, wk)
            load_w(WVT, wv)
            load_w(WQT, wq)
            for tch in range(16):
                ocol = vt[tch][:].rearrange("p (h e) -> p h e", h=HPG)[:, :, 64:65]
                nc.vector.memset(ocol, 1.0)

            def k_tb(t, xts):
                psums = {och: psA.tile([128, 512], F32,
                                       tag=f"pj{och}_0",
                                       name=f"pj{och}_0")
                         for och in range(4)}
                for i in range(IC):
                    for och in range(4):
                        nc.tensor.matmul(
                            psums[och], wk[i][:, och * 128:(och + 1) * 128],
                            xts[i][:], start=(i == 0), stop=(i == IC - 1))
                for och in range(4):
                    nc.scalar.activation(
                        kt[och][:, t * 512:(t + 1) * 512], psums[och],
                        mybir.ActivationFunctionType.Identity,
                        bias=bk_t[:, och:och + 1])

            def v_rnd(vr, xts):
                psums = {j: psA.tile(
                    [128, 512], F32, tag=f"pj{j}_1",
                    name=f"pv{j}") for j in range(4)}
                for i in range(IC):
                    for j in range(4):
                        nc.tensor.matmul(
                            psums[j], xts[i][:, j * 128:(j + 1) * 128],
                            wv[i][:], start=(i == 0), stop=False)
                for j in range(4):
                    tch = 4 * vr + j
                    # bias row: ones[t] (x) bv[o], closes the accum group
                    nc.tensor.matmul(
                        psums[j], ones_t[0:1, 0:128], bv_t[0:1, :],
                        start=False, stop=True)
                    nc.vector.tensor_copy(
                        vt[tch][:].rearrange(
                            "p (h e) -> p h e", h=HPG)[:, :, 0:64],
                        psums[j][:].rearrange("p (h d) -> p h d", h=HPG))

            xqts = {}

            def q_och(t, och, ps, evac_act=False):
                """Q projection for one (t-block, och): 8 MMs + evac."""
                for i in range(IC):
                    nc.tensor.matmul(
                        ps, wq[i][:, och * 128:(och + 1) * 128],
                        xqts[t][i][:], start=(i == 0), stop=(i == IC - 1))
                if evac_act:
                    nc.scalar.activation(
                        qtt[(och, t)][:], ps,
                        mybir.ActivationFunctionType.Identity,
                        bias=bq_t[:, och:och + 1])
                else:
                    nc.vector.tensor_scalar_add(
                        qtt[(och, t)][:], ps, bq_t[:, och:och + 1])

            # interleave K t-blocks and V rounds with x loads prefetched two
            # rounds ahead; Q t-block 0 before the last K/V round so
            # attention starts as early as possible
            rounds = [("k", 0), ("v", 0), ("k", 1), ("v", 1), ("k", 2),
                      ("v", 2), ("q", 0), ("k", 3), ("v", 3)]
            RSRC = {"k": XKT, "v": XVT, "q": XQT}
            xrs = [load_xtb(RSRC[rounds[0][0]], rounds[0][1]),
                   load_xtb(RSRC[rounds[1][0]], rounds[1][1])]
            for r, (kind, idx) in enumerate(rounds):
                if r + 2 < len(rounds):
                    k2, i2 = rounds[r + 2]
                    xrs.append(load_xtb(RSRC[k2], i2))
                xts = xrs[r]
                if kind == "k":
                    k_tb(idx, xts)
                elif kind == "v":
                    v_rnd(idx, xts)
                else:
                    xqts[0] = xts
                    for och in range(4):
                        q_och(0, och,
                              psA.tile([128, 512], F32, tag=f"pj{och}_0",
                                       name=f"q0_{och}")[:], evac_act=True)
            psA.release()

            # ---------------- Phase B/C: attention + out-proj ----------------
            with tc.tile_pool(name="wopool", bufs=1) as wop, \
                 tc.tile_pool(name="mpool", bufs=2) as mp, \
                 tc.tile_pool(name="ptpool", bufs=2) as ptp, \
                 tc.tile_pool(name="wkpool", bufs=2) as wkp, \
                 tc.tile_pool(name="onrmpool", bufs=2) as onp, \
                 tc.tile_pool(name="stgpool", bufs=2) as sgp, \
                 tc.tile_pool(name="psB", bufs=1, space="PSUM") as psB:
                wo = [wop.tile([128, D_MODEL], F16, name=f"wo{j}")
                      for j in range(4)]
                for j in range(4):
                    nc.sync.dma_start(out=wo[j][:],
                                      in_=WOT[j * 128:(j + 1) * 128, :])
                def load_mask(t):
                    # 8 fat DMAs on sync/gpsimd: SWDGE issue costs ~1us on
                    # the issuing engine; never the scalar queue (blocks ACT)
                    mh = mp.tile([128, 16 * 512], F16, tag="mask", name="mask_t")
                    msrc = MASKT.rearrange("(s8 p) c -> p s8 c", p=128)
                    for k in range(8):
                        nc.sync.dma_start(
                            out=mh[:, k * 1024:(k + 1) * 1024].rearrange(
                                "p (s2 c) -> p s2 c", s2=2),
                            in_=msrc[:, 2 * k:2 * k + 2,
                                     t * 512:(t + 1) * 512])
                    return mh

                # software-pipelined: head state flows one step behind
                tT_holder = [None]

                def attn_u(state, u, acc, half=None):
                    """attnV accumulation u-group (16 MMs on PE); half=0/1
                    emits 8 MMs so one blob never overflows the 32-deep PE
                    exec queue. u-groups are strictly sequential per psum
                    tile — interleaved starts mis-accumulate on HW."""
                    h, pts = state
                    scs = range(16) if half is None else \
                        range(8 * half, 8 * half + 8)
                    for sc in scs:
                        pt = pts[sc // 4]
                        off = (sc % 4) * 512 + u * 128
                        nc.tensor.matmul(
                            acc[:, u * 65:(u + 1) * 65],
                            pt[:, off:off + 128],
                            vt[sc][:, h * 65:(h + 1) * 65],
                            start=(sc == 0), stop=(sc == 15),
                            skip_group_check=True)

                def attn_norm(state, acc):
                    """reciprocal + normalize (DVE only)."""
                    rc = wkp.tile([128, 4], F32, tag="rc", name="rc")
                    att = wkp.tile([128, 256], F16, tag="att", name="att")
                    nc.vector.reciprocal(
                        rc[:],
                        acc[:].rearrange("p (q e) -> p q e", q=4)[:, :, 64])
                    for u in range(4):
                        nc.vector.tensor_scalar_mul(
                            att[:, u * 64:(u + 1) * 64],
                            acc[:, u * 65:u * 65 + 64], rc[:, u:u + 1])
                    return att

                def attn_transp(state, att, onrm):
                    """transpose [t,d]->[d,t] + onrm copy (PE + DVE)."""
                    h, _ = state
                    if h % 2 == 0:
                        tT_holder[0] = psB.tile([128, 512], F16, tag="tT",
                                                name="tT")
                    tT = tT_holder[0]
                    ho = (h % 2) * 64
                    for u in range(4):
                        nc.tensor.transpose(
                            tT[ho:ho + 64, u * 128:(u + 1) * 128],
                            att[:, u * 64:(u + 1) * 64], id_t[:])
                    if h % 2 == 1:
                        nc.vector.tensor_copy(onrm[h // 2][:], tT[:])

                sring = [0]  # rotating S0/S1/S2 psum ring counter

                def s_tile():
                    i = sring[0] % 3
                    sring[0] += 1
                    return psB.tile([128, 1024], F32, tag=f"S{i}",
                                    name=f"S{i}")

                def out_proj(t, onrm, opi):
                    opt = s_tile()
                    for col in range(2):
                        och = opi * 2 + col
                        for cch in range(4):
                            nc.tensor.matmul(
                                opt[:, col * 512:(col + 1) * 512],
                                wo[cch][:, och * 128:(och + 1) * 128],
                                onrm[cch][:], start=(cch == 0), stop=(cch == 3))
                    stg = sgp.tile([128, 1024], F32, tag="stg", name="stg")
                    nc.vector.tensor_copy(stg[:], opt[:])
                    nc.sync.dma_start(
                        out=OUTT.rearrange("(j p) m -> p j m", p=128)
                        [:, opi * 2:opi * 2 + 2, t * 512:(t + 1) * 512],
                        in_=stg[:].rearrange("p (j m) -> p j m", j=2))

                mh = load_mask(0)
                mm_state = None     # head awaiting attnV u-groups 0-2
                u3_state = None     # (state, acc) awaiting u3 + norm
                fin_state = None    # (state, att) awaiting transpose
                acc_cur = None
                pending_op = None   # (t, onrm) awaiting out-projection
                onrm = None
                for t in range(4):
                    mh_next = None
                    onrm_prev, onrm = onrm, [
                        onp.tile([128, 512], F16, tag=f"onrm{j}",
                                 name=f"onrm{j}") for j in range(4)]
                    for h in range(HPG):
                        ht, ho = h // 2, (h % 2) * 64
                        pts = []
                        for ci in range(NCHUNK):
                            ps = s_tile()
                            if ci % 2 == 0:
                                pt = ptp.tile([128, 2048], F16,
                                              tag=f"p{ci // 2}",
                                              name=f"p{ci // 2}")
                                pts.append(pt)
                            else:
                                pt = pts[-1]
                            for i in range(2):
                                sc = 2 * ci + i
                                nc.tensor.matmul(
                                    ps[:, i * 512:(i + 1) * 512],
                                    kt[ht][ho:ho + 64, sc * 128:(sc + 1) * 128],
                                    qtt[(ht, t)][ho:ho + 64, :],
                                    start=True, stop=True)
                            nc.scalar.activation(
                                pt[:, (ci % 2) * 1024:(ci % 2 + 1) * 1024],
                                ps, Exp, scale=0.125)
                            if ci % 2 == 1:
                                # mask multiply over the 2048-wide pair; Pool
                                # only ever gets pair 1 (it is slow and the
                                # later pairs are latency-critical)
                                pair = ci // 2
                                meng = nc.gpsimd if pair == 1 else nc.vector
                                meng.tensor_mul(
                                    pt[:], pt[:],
                                    mh[:, pair * 2048:(pair + 1) * 2048])
                            # deferred tails AFTER exp/mask so the DVE queue
                            # issues masks ahead of the norm chain
                            if ci == 0 and u3_state is not None:
                                st_u, acc_u = u3_state
                                attn_u(st_u, 3, acc_u, half=0)
                            if ci == 1 and u3_state is not None:
                                st_u, acc_u = u3_state
                                attn_u(st_u, 3, acc_u, half=1)
                                att_n = attn_norm(st_u, acc_u)
                                fin_state = (st_u, att_n)
                                u3_state = None
                            if ci == 2 and pending_op is not None \
                                    and 2 <= h <= 5:
                                # onrm[3] of prev t lands at h1.ci3; start at h2
                                out_proj(pending_op[0], pending_op[1], h - 2)
                                if h == 5:
                                    pending_op = None
                            if t == 0 and h in (0, 1, 3) and ci == 4:
                                # prefetch the next Q t-block's x chunks
                                xqts[{0: 1, 1: 2, 3: 3}[h]] = load_xtb(
                                    XQT, {0: 1, 1: 2, 3: 3}[h])
                            if t == 0 and 1 <= h <= 6 and ci in (2, 6):
                                # stream one deferred Q och-block (8 MMs)
                                # through the ring per slot
                                k = (h - 1) * 2 + (ci == 6)
                                qtb, qoch = 1 + k // 4, k % 4
                                sq = s_tile()
                                q_och(qtb, qoch, sq[:, 0:512])
                            if ci == 3 and fin_state is not None:
                                # transposes (norm from ci0 is done by now)
                                st_f, att_f = fin_state
                                attn_transp(st_f, att_f,
                                            onrm if st_f[0] != 7 else onrm_prev)
                                fin_state = None
                            if ci >= 2 and mm_state is not None:
                                # previous head's attnV, one 8-MM half-group
                                # per chunk slot (masks are long done by now)
                                if ci == 2:
                                    acc_cur = psB.tile([128, 4 * 65], F32,
                                                       tag="acc", name="acc")
                                attn_u(mm_state, (ci - 2) // 2, acc_cur,
                                       half=(ci - 2) % 2)
                        if mm_state is not None:
                            u3_state = (mm_state, acc_cur)
                        mm_state = (h, pts)
                        if h == 4 and t < 3:
                            mh_next = load_mask(t + 1)
                    if t < 3:
                        pending_op = (t, onrm)
                        mh = mh_next
                # drain: finish heads 6 and 7 of t3, then out-projections
                st_u, acc_u = u3_state
                attn_u(st_u, 3, acc_u)
                attn_transp(st_u, attn_norm(st_u, acc_u), onrm)
                acc_cur = psB.tile([128, 4 * 65], F32, tag="acc", name="acc")
                for u in range(4):
                    attn_u(mm_state, u, acc_cur)
                attn_transp(mm_state, attn_norm(mm_state, acc_cur), onrm)
                for opi in range(4):
                    out_proj(3, onrm, opi)

            xp.release()
            wp.release()
            vtp.release()

    nc.compile()
    return nc


_NC_CACHE = {}


def _get_module():
    if "nc" not in _NC_CACHE:
        _NC_CACHE["nc"] = build_module()
    return _NC_CACHE["nc"]


def kernel(q, k, v, mask, Wq, bq, Wk, bk, Wv, bv, Wo, bo, **_ignored):
    q = np.asarray(q, dtype=np.float32)
    k = np.asarray(k, dtype=np.float32)
    v = np.asarray(v, dtype=np.float32)
    mask = np.asarray(mask)
    Wq, Wk, Wv, Wo = (np.asarray(w, dtype=np.float32) for w in (Wq, Wk, Wv, Wo))
    bq, bk, bv, bo = (np.asarray(b_, dtype=np.float32) for b_ in (bq, bk, bv, bo))

    maskT = (np.ascontiguousarray(mask[0, 0].T) != 0).astype(np.float16)
    idm = np.eye(128, dtype=np.float16)

    xT = {}
    for b_ in range(B):
        xT[("q", b_)] = np.ascontiguousarray(q[b_].T).astype(np.float16)
        xT[("k", b_)] = np.ascontiguousarray(k[b_].T).astype(np.float16)
        xT[("v", b_)] = np.ascontiguousarray(v[b_].T).astype(np.float16)
    wslice = {}
    for hg in range(2):
        og = hg * OG
        wslice[("q", hg)] = np.ascontiguousarray(Wq[og:og + OG, :].T).astype(np.float16)
        wslice[("k", hg)] = np.ascontiguousarray(Wk[og:og + OG, :].T).astype(np.float16)
        wslice[("v", hg)] = np.ascontiguousarray(Wv[og:og + OG, :].T).astype(np.float16)
        wslice[("o", hg)] = np.ascontiguousarray(Wo[:, og:og + OG].T).astype(np.float16)

    in_maps = []
    for c in range(NCORES):
        b_, hg = c // 2, c % 2
        og = hg * OG
        in_maps.append({
            "XQT": xT[("q", b_)], "XKT": xT[("k", b_)], "XVT": xT[("v", b_)],
            "WQT": wslice[("q", hg)], "WKT": wslice[("k", hg)],
            "WVT": wslice[("v", hg)], "WOT": wslice[("o", hg)],
            "MASKT": maskT,
            "BQ": bq[og:og + OG].astype(np.float32),
            "BK": bk[og:og + OG].astype(np.float32),
            "BV": bv[og:og + OG].reshape(1, OG).astype(np.float16),
            "IDT": idm,
        })

    nc = _get_module()
    res = run_bass_kernel_spmd(nc, in_maps, list(range(NCORES)))

    out = np.empty((B, S, D_MODEL), np.float32)
    for b_ in range(B):
        acc = res.results[2 * b_]["OUTT"] + res.results[2 * b_ + 1]["OUTT"]
        out[b_] = acc.T + bo
    return out
